# revision 1
# baseline (speedup 1.0000x reference)
"""CSR Linear kernel for TRN2: out = x @ W^T + bias, W from COO nonzeros.

Strategy: data-parallel over tokens across 8 NeuronCores. Host densifies the
sparse weight into WT[in, out] (duplicate coords summed) and transposes x;
each core computes its 1024-token shard with a tiled f32r (TF32) matmul:
WT streamed from HBM once, x^T resident in SBUF, bias fused into the
PSUM->SBUF eviction.
"""

import os
import sys
import types

import numpy as np

TOKENS = 8192
IN_F = 4096
OUT_F = 4096
N_CORES = 8
P = 128

_CACHE = {}


def _ensure_ntff_hook():
    """Register the axon NTFF profile hook if the antenv stub lacks it.

    Only needed when tracing (BASS_TRACE=1); harmless otherwise. In
    environments with a real antenv.axon_hooks this is a no-op.
    """
    try:
        import antenv.axon_hooks  # noqa: F401

        return
    except ImportError:
        pass
    try:
        import antenv
        from trn_agent_boot.trn_boot import _ntff_profile_via_ctypes

        hooks = types.ModuleType("antenv.axon_hooks")
        hooks._hook = _ntff_profile_via_ctypes("/opt/axon/libaxon_pjrt.so")
        hooks.set_axon_ntff_profile_hook = lambda h: setattr(hooks, "_hook", h)
        hooks.get_axon_ntff_profile_hook = lambda: hooks._hook
        sys.modules["antenv.axon_hooks"] = hooks
        antenv.axon_hooks = hooks
    except Exception:
        pass


def _patch_upload():
    """Make trace artifact upload fall back to the local tmpdir when no
    artifact bucket is reachable (container environments)."""
    from concourse import bass_utils

    orig = bass_utils.upload_artifacts
    if getattr(orig, "_kernel_patched", False):
        return

    def _safe_upload(tmpdir):
        try:
            return orig(tmpdir)
        except Exception:
            return tmpdir

    _safe_upload._kernel_patched = True
    bass_utils.upload_artifacts = _safe_upload


def build_program(tok_per_core=TOKENS // N_CORES, in_f=IN_F, out_f=OUT_F):
    """Build + compile the per-core Bass program.

    out[tok_per_core, out_f] = xt.T @ wt + bias, with
      xt [in_f, tok_per_core] (f32r), wt [in_f, out_f] (f32r),
      biasr [1, out_f] (f32, partition-broadcast on the PE at startup).
    """
    key = (tok_per_core, in_f, out_f)
    if key in _CACHE:
        return _CACHE[key]

    import concourse.bacc as bacc
    import concourse.mybir as mybir
    import concourse.tile as tile

    N_TILE = 512  # out-feature block per psum bank
    KO = in_f // P  # k tiles
    M = tok_per_core // P  # token tiles
    NB = out_f // N_TILE  # out-feature blocks
    KO_CHUNK = 4  # k-tiles per WT DMA (1 MiB transfers)

    nc = bacc.Bacc("TRN2", target_bir_lowering=False, debug=False)

    xt = nc.dram_tensor("xt", [in_f, tok_per_core], mybir.dt.float32r, kind="ExternalInput")
    wt = nc.dram_tensor("wt", [in_f, out_f], mybir.dt.float32r, kind="ExternalInput")
    biasr = nc.dram_tensor("biasr", [1, out_f], mybir.dt.float32, kind="ExternalInput")
    out = nc.dram_tensor("out", [tok_per_core, out_f], mybir.dt.float32, kind="ExternalOutput")

    xt_ap = xt.ap().rearrange("(ko p) t -> p ko t", p=P)  # [P, KO, T]
    wt_ap = wt.ap().rearrange("(ko p) o -> p ko o", p=P)  # [P, KO, out_f]
    out_ap = out.ap().rearrange("(mo p) o -> p mo o", p=P)  # [P, M, out_f]

    with tile.TileContext(nc) as tc:
        WT_BUFS = 6
        with (
            tc.tile_pool(name="xt_pool", bufs=1) as xt_pool,
            tc.tile_pool(name="bias_pool", bufs=1) as bias_pool,
            tc.tile_pool(name="wt_pool", bufs=WT_BUFS) as wt_pool,
            tc.tile_pool(name="out_pool", bufs=4) as out_pool,
            tc.tile_pool(name="psum", bufs=8, space="PSUM") as psum_pool,
        ):
            xt_sb = xt_pool.tile([P, KO, tok_per_core], mybir.dt.float32r)

            def bounds(first, step):
                b = [0, min(first, KO)]
                while b[-1] + step < KO:
                    b.append(b[-1] + step)
                if b[-1] < KO:
                    b.append(KO)
                return list(zip(b[:-1], b[1:]))

            # Small leading chunks so the first matmul can start early.
            wt_chunks = {n: bounds(2 if n == 0 else KO_CHUNK, KO_CHUNK) for n in range(NB)}
            xt_chunks = bounds(1, min(KO, max(1, (1 << 20) // (P * tok_per_core * 4))))

            def load_wt(n, kb, kbe):
                ns = slice(n * N_TILE, (n + 1) * N_TILE)
                wt_t = wt_pool.tile(
                    [P, KO_CHUNK, N_TILE],
                    mybir.dt.float32r,
                    name=f"wt_{n}_{kb}",
                    tag="wt",
                )
                nc.sync.dma_start(wt_t[:, : kbe - kb, :], wt_ap[:, kb:kbe, ns])
                return wt_t

            def load_xt(j, je):
                return nc.sync.dma_start(xt_sb[:, j:je, :], xt_ap[:, j:je, :])

            # Interleave the first n-block's WT chunks 1:2 with x^T chunks —
            # the PE consumes x^T bytes at ~2x the WT rate in the first block.
            preloaded = {}
            xi = 0
            bias_emitted = False
            bias_sb = bias_pool.tile([P, out_f], mybir.dt.float32)

            def emit_bias_broadcast():
                # Broadcast the [1, out_f] bias across all 128 partitions on
                # the PE during the otherwise-idle startup window: with
                # onehot0[k, m] = (k == 0), psum[m, o] = sum_k onehot0[k, m] *
                # brow[k, o] = bias[o]. Saves shipping a 2 MB replicated bias
                # through the bandwidth-critical first n-block. f32r operands
                # come from DVE rounding copies (memset can't write f32r).
                ones_f = bias_pool.tile([P, P], mybir.dt.float32)
                nc.gpsimd.memset(ones_f[:], 0.0)
                nc.gpsimd.memset(ones_f[0:1, :], 1.0)
                ones_t = bias_pool.tile([P, P], mybir.dt.float32r)
                nc.vector.tensor_copy(out=ones_t[:], in_=ones_f[:])
                for b in range(NB):
                    bs = slice(b * N_TILE, (b + 1) * N_TILE)
                    brow_f = out_pool.tile(
                        [P, N_TILE], mybir.dt.float32, name=f"browf_{b}", tag="ot"
                    )
                    nc.gpsimd.memset(brow_f[:], 0.0)
                    nc.sync.dma_start(brow_f[0:1, :], biasr.ap()[:, bs])
                    brow_r = out_pool.tile(
                        [P, N_TILE], mybir.dt.float32r, name=f"browr_{b}", tag="ot"
                    )
                    nc.vector.tensor_copy(out=brow_r[:], in_=brow_f[:])
                    pb = psum_pool.tile(
                        [P, N_TILE], mybir.dt.float32, name=f"bias_ps_{b}", tag="ps"
                    )
                    nc.tensor.matmul(
                        pb[:], lhsT=ones_t[:], rhs=brow_r[:], start=True, stop=True
                    )
                    nc.vector.tensor_copy(out=bias_sb[:, bs], in_=pb[:])

            for kb, kbe in wt_chunks[0][:WT_BUFS]:
                preloaded[(0, kb)] = load_wt(0, kb, kbe)
                for _ in range(2):
                    if xi < len(xt_chunks):
                        load_xt(*xt_chunks[xi])
                        xi += 1
                if not bias_emitted:
                    # After the first wt+xt pair so the tiny bias DMAs don't
                    # delay the chunk DMAs that gate the first real matmul.
                    emit_bias_broadcast()
                    bias_emitted = True
            for j, je in xt_chunks[xi:]:
                load_xt(j, je)

            for n in range(NB):
                ns = slice(n * N_TILE, (n + 1) * N_TILE)
                ps = [
                    psum_pool.tile(
                        [P, N_TILE], mybir.dt.float32, name=f"ps_{n}_{m}", tag="ps"
                    )
                    for m in range(M)
                ]
                for kb, kbe in wt_chunks[n]:
                    wt_t = preloaded.pop((n, kb), None)
                    if wt_t is None:
                        wt_t = load_wt(n, kb, kbe)
                    for kk in range(kbe - kb):
                        ko = kb + kk
                        for m in range(M):
                            nc.tensor.matmul(
                                ps[m][:],
                                lhsT=xt_sb[:, ko, m * P : (m + 1) * P],
                                rhs=wt_t[:, kk, :],
                                start=(ko == 0),
                                stop=(ko == KO - 1),
                            )
                for m in range(M):
                    ot = out_pool.tile(
                        [P, N_TILE], mybir.dt.float32, name=f"ot_{n}_{m}", tag="ot"
                    )
                    nc.vector.tensor_add(out=ot[:], in0=ps[m][:], in1=bias_sb[:, ns])
                    nc.sync.dma_start(out_ap[:, m, ns], ot[:])

    nc.compile()
    _CACHE[key] = nc
    return nc


def _densify_wt(values, row_ids, col_ids, in_f=IN_F, out_f=OUT_F):
    """WT[i, o] = sum of values[k] over k with col_ids[k]==i, row_ids[k]==o."""
    idx = col_ids.astype(np.int64) * out_f + row_ids.astype(np.int64)
    wt = np.bincount(idx, weights=values.astype(np.float64), minlength=in_f * out_f)
    return np.ascontiguousarray(wt.astype(np.float32).reshape(in_f, out_f))


def kernel(x, values, row_ids, col_ids, bias):
    from concourse import bass_utils

    if os.environ.get("BASS_TRACE"):
        _ensure_ntff_hook()
        _patch_upload()

    nc = build_program()

    x = np.asarray(x, dtype=np.float32)
    values = np.asarray(values, dtype=np.float32)
    row_ids = np.asarray(row_ids)
    col_ids = np.asarray(col_ids)
    bias = np.asarray(bias, dtype=np.float32)

    wt = _densify_wt(values, row_ids, col_ids)
    bias_rep = np.ascontiguousarray(bias.astype(np.float32)[None, :])
    tpc = TOKENS // N_CORES
    in_maps = []
    for c in range(N_CORES):
        xt_c = np.ascontiguousarray(x[c * tpc : (c + 1) * tpc, :].T)
        in_maps.append({"xt": xt_c, "wt": wt, "biasr": bias_rep})

    res = bass_utils.run_bass_kernel_spmd(nc, in_maps, core_ids=list(range(N_CORES)))
    global last_results
    last_results = res
    return np.concatenate([res.results[c]["out"] for c in range(N_CORES)], axis=0)


last_results = None



# revision 2
# speedup vs baseline: 1.1115x; 1.1115x over previous
"""CSR Linear kernel for TRN2: out = x @ W^T + bias, W from COO nonzeros.

Strategy: data-parallel over tokens across 8 NeuronCores. Host densifies the
sparse weight into A[in, out] (duplicate coords summed) in bf16; each core
computes its 1024-token shard as out^T = A^T-tiles stationary on the PE with
x^T streaming:  psum[128 outf, 512 tok] += A_tile[128 k, 128 outf].T @
xT[128 k, 512 tok].  With out-features on PSUM partitions the bias add is a
per-partition tensor_scalar on the eviction. bf16 operands halve DMA bytes
and enable fast weight load; phase A runs the first 4 out-tiles k-outer so
DMA demand stays under the per-core HBM rate from the first matmul, phase B
runs o-major k-sweeps at pure PE rate with per-sweep evictions.
"""

import os
import sys
import types

import ml_dtypes
import numpy as np

TOKENS = 8192
IN_F = 4096
OUT_F = 4096
N_CORES = 8
P = 128

_CACHE = {}


def _ensure_ntff_hook():
    """Register the axon NTFF profile hook if the antenv stub lacks it.

    Only needed when tracing (BASS_TRACE=1); harmless otherwise. In
    environments with a real antenv.axon_hooks this is a no-op.
    """
    try:
        import antenv.axon_hooks  # noqa: F401

        return
    except ImportError:
        pass
    try:
        import antenv
        from trn_agent_boot.trn_boot import _ntff_profile_via_ctypes

        hooks = types.ModuleType("antenv.axon_hooks")
        hooks._hook = _ntff_profile_via_ctypes("/opt/axon/libaxon_pjrt.so")
        hooks.set_axon_ntff_profile_hook = lambda h: setattr(hooks, "_hook", h)
        hooks.get_axon_ntff_profile_hook = lambda: hooks._hook
        sys.modules["antenv.axon_hooks"] = hooks
        antenv.axon_hooks = hooks
    except Exception:
        pass


def _patch_upload():
    """Make trace artifact upload fall back to the local tmpdir when no
    artifact bucket is reachable (container environments)."""
    from concourse import bass_utils

    orig = bass_utils.upload_artifacts
    if getattr(orig, "_kernel_patched", False):
        return

    def _safe_upload(tmpdir):
        try:
            return orig(tmpdir)
        except Exception:
            return tmpdir

    _safe_upload._kernel_patched = True
    bass_utils.upload_artifacts = _safe_upload


def build_program(tok_per_core=TOKENS // N_CORES, in_f=IN_F, out_f=OUT_F):
    """Build + compile the per-core Bass program.

    outT[out_f, tok_per_core] = sum_k A[k, :].T-tiles @ xT[k, tokens] + bias
    with A [in_f, out_f] bf16 (host-densified W^T), xT [in_f, tok] bf16.
    """
    key = (tok_per_core, in_f, out_f)
    if key in _CACHE:
        return _CACHE[key]

    import concourse.bacc as bacc
    import concourse.mybir as mybir
    import concourse.tile as tile

    KO = in_f // P  # 32 contraction tiles
    NB = out_f // P  # 32 out-feature tiles
    NH = tok_per_core // 512  # 2 token halves (psum bank = 512 f32)
    A_TILES = 4  # phase-A out-tiles (k-outer), 4*NH = 8 psum banks

    nc = bacc.Bacc("TRN2", target_bir_lowering=False, debug=False)

    # xt2[p, ko*T + t] = x_shard^T[ko*128+p, t]
    xt = nc.dram_tensor("xt", [P, KO * tok_per_core], mybir.dt.bfloat16, kind="ExternalInput")
    # wt2[nb*128+p, ko*128+o] = A[ko*128+p, nb*128+o]
    wt = nc.dram_tensor("wt", [out_f, in_f], mybir.dt.bfloat16, kind="ExternalInput")
    # biasr[p, nb] = bias[nb*128+p]
    biasr = nc.dram_tensor("biasr", [P, NB], mybir.dt.float32, kind="ExternalInput")
    # outT[nb*128+p, t] = out[t, nb*128+p]
    out = nc.dram_tensor("out", [out_f, tok_per_core], mybir.dt.float32, kind="ExternalOutput")

    xt_ap = xt.ap().rearrange("p (ko t) -> p ko t", ko=KO)
    wt_ap = wt.ap().rearrange("(nb p) (ko o) -> p nb ko o", p=P, o=P)
    out_ap = out.ap().rearrange("(nb p) t -> p nb t", p=P)

    with tile.TileContext(nc) as tc:
        with (
            tc.tile_pool(name="xt_pool", bufs=1) as xt_pool,
            tc.tile_pool(name="bias_pool", bufs=1) as bias_pool,
            tc.tile_pool(name="wt_pool", bufs=6) as wt_pool,
            tc.tile_pool(name="out_pool", bufs=4) as out_pool,
            tc.tile_pool(name="psum", bufs=8, space="PSUM") as psum_pool,
        ):
            xt_sb = xt_pool.tile([P, KO, tok_per_core], mybir.dt.bfloat16)
            bias_sb = bias_pool.tile([P, NB], mybir.dt.float32)
            nc.sync.dma_start(bias_sb[:], biasr.ap())

            wt_tiles = {}

            def wt_tile(o):
                if o not in wt_tiles:
                    wt_tiles[o] = wt_pool.tile(
                        [P, KO, P], mybir.dt.bfloat16, name=f"wt_{o}", tag="wt"
                    )
                return wt_tiles[o]

            def load_wt(o, kb, kbe):
                nc.sync.dma_start(wt_tile(o)[:, kb:kbe, :], wt_ap[:, o, kb:kbe, :])

            def load_xt(kb, kbe):
                nc.sync.dma_start(xt_sb[:, kb:kbe, :], xt_ap[:, kb:kbe, :])

            def evict(o, h, ps):
                ot = out_pool.tile([P, 512], mybir.dt.float32, name=f"ot_{o}_{h}", tag="ot")
                nc.vector.tensor_scalar_add(ot[:], ps[:], bias_sb[:, o : o + 1])
                nc.sync.dma_start(out_ap[:, o, h * 512 : (h + 1) * 512], ot[:])

            # ---- Phase A: out-tiles 0..3, k-outer so DMA demand is smooth ----
            # chunks sized fine at the start so the first matmul gates on
            # ~0.3 MiB of DMA, coarser later.
            chunks = [(0, 1), (1, 4), (4, 8), (8, 16), (16, 24), (24, 32)]
            ps_a = {
                (o, h): psum_pool.tile([P, 512], mybir.dt.float32, name=f"psA_{o}_{h}", tag="ps")
                for o in range(A_TILES)
                for h in range(NH)
            }
            for ci, (kb, kbe) in enumerate(chunks):
                load_xt(kb, kbe)
                for o in range(A_TILES):
                    load_wt(o, kb, kbe)
                if ci == len(chunks) - 2:
                    load_wt(A_TILES, 0, KO)  # phase-B prefetch into spare bufs
                if ci == len(chunks) - 1:
                    load_wt(A_TILES + 1, 0, KO)
            for kb, kbe in chunks:
                for o in range(A_TILES):
                    wto = wt_tile(o)
                    for h in range(NH):
                        for ko in range(kb, kbe):
                            nc.tensor.matmul(
                                ps_a[(o, h)][:],
                                lhsT=wto[:, ko, :],
                                rhs=xt_sb[:, ko, h * 512 : (h + 1) * 512],
                                start=(ko == 0),
                                stop=(ko == KO - 1),
                            )
            for o in range(A_TILES):
                for h in range(NH):
                    evict(o, h, ps_a[(o, h)])

            # ---- Phase B: o-major k-sweeps at pure PE rate ----
            for o in range(A_TILES, NB):
                if o + 2 < NB:
                    load_wt(o + 2, 0, KO)
                wto = wt_tile(o)
                for h in range(NH):
                    ps = psum_pool.tile(
                        [P, 512], mybir.dt.float32, name=f"ps_{o}_{h}", tag="ps"
                    )
                    for ko in range(KO):
                        nc.tensor.matmul(
                            ps[:],
                            lhsT=wto[:, ko, :],
                            rhs=xt_sb[:, ko, h * 512 : (h + 1) * 512],
                            start=(ko == 0),
                            stop=(ko == KO - 1),
                        )
                    evict(o, h, ps)

    nc.compile()
    _CACHE[key] = nc
    return nc


def _densify_a(values, row_ids, col_ids, in_f=IN_F, out_f=OUT_F):
    """A[i, o] = sum of values[k] over k with col_ids[k]==i, row_ids[k]==o."""
    idx = col_ids.astype(np.int64) * out_f + row_ids.astype(np.int64)
    a = np.bincount(idx, weights=values.astype(np.float64), minlength=in_f * out_f)
    return a.astype(np.float32).reshape(in_f, out_f)


def kernel(x, values, row_ids, col_ids, bias):
    from concourse import bass_utils

    if os.environ.get("BASS_TRACE"):
        _ensure_ntff_hook()
        _patch_upload()

    nc = build_program()

    x = np.asarray(x, dtype=np.float32)
    values = np.asarray(values, dtype=np.float32)
    row_ids = np.asarray(row_ids)
    col_ids = np.asarray(col_ids)
    bias = np.asarray(bias, dtype=np.float32)

    KO = IN_F // P
    NB = OUT_F // P
    tpc = TOKENS // N_CORES

    a = _densify_a(values, row_ids, col_ids)  # [in_f, out_f] f32
    # wt2[nb, p, ko, o] = A[ko*128+p, nb*128+o]
    wt2 = np.ascontiguousarray(
        a.reshape(KO, P, NB, P).transpose(2, 1, 0, 3).reshape(OUT_F, IN_F)
    ).astype(ml_dtypes.bfloat16)
    bias2 = np.ascontiguousarray(bias.reshape(NB, P).T).astype(np.float32)

    in_maps = []
    for c in range(N_CORES):
        xT = x[c * tpc : (c + 1) * tpc, :].T  # [in_f, tpc]
        xt2 = np.ascontiguousarray(
            xT.reshape(KO, P, tpc).transpose(1, 0, 2).reshape(P, KO * tpc)
        ).astype(ml_dtypes.bfloat16)
        in_maps.append({"xt": xt2, "wt": wt2, "biasr": bias2})

    res = bass_utils.run_bass_kernel_spmd(nc, in_maps, core_ids=list(range(N_CORES)))
    global last_results
    last_results = res
    return np.ascontiguousarray(
        np.concatenate([res.results[c]["out"].T for c in range(N_CORES)], axis=0)
    )


last_results = None
